# revision 5
# baseline (speedup 1.0000x reference)
"""CLIP-style contrastive loss on 8 Trainium2 NeuronCores.

Math: with labels = eye(B), the Keras CCE(prob, eye) loss only depends on the
diagonal of the softmax probabilities:
    sum_ij prob_ij * logclip_ij = tr * log(1-eps) + (B - tr) * log(eps)
where tr = trace(prob). Since |logits| <= exp(temperature) (cosine
similarities), softmax needs no max subtraction; prob_ii = E_ii / rowsum(E)
with E = exp(scale * S), S = l2norm(left) @ l2norm(right).T.

Sharding: 4x2 grid over the BxB similarity matrix. Core (p, q) owns
left rows [1024p, 1024p+1024) x right rows [2048q, 2048q+2048).

Per-core pipeline (v2):
  - 24 row-tile loads (fp32, SWDGE), square+sqsum on ScalarE (accum_out),
    inv-norm via exp(-0.5*ln(sq+eps)) so Square/Ln/Exp share ONE ACT table
    set (natural_log_exp_and_others; the old Sqrt cost 7 table loads = 9us).
  - normalize+cast to bf16 on VectorE, one xbar DMA transpose PER ROW TILE:
    L tiles go tile-major (lnT[:, mt, k, :] = stationary), R tiles go
    K-MAJOR (rnT[:, k, rt, :]) so the moving operand of each matmul is a
    contiguous [128, 512] slice.
  - MM stream ordered h -> mt -> k -> j with a 3-deep [128,1024] PSUM
    rotation: back-to-back matmuls keep the PE HAM-warm; exp+rowsum fuse
    into one ScalarE pass per PSUM tile, writing E in fp8e4 (halves SBUF
    and the colsum stream; precision irrelevant at 2e-2 tolerance).
  - column sums via ones-vector fp8 matmuls interleaved one-mt-behind the
    main stream (no PE stall); diagonal via fused tensor_tensor_reduce.
Host combines partial sums (O(B) work only).
"""

import math
import numpy as np

import concourse.bass as bass
import concourse.mybir as mybir
import concourse.tile as tile
from concourse import bacc
from concourse.bass import ds, ts
from concourse.masks import make_identity

B = 4096
D = 1024
EPS = 1e-7
WEIGHT = 1.0

PGRID = 4  # row groups (left)
QGRID = 2  # col groups (right)
LROWS = B // PGRID   # 1024 left rows per core
RROWS = B // QGRID   # 2048 right rows per core
KT = D // 128        # 8 contraction k-tiles
MT = LROWS // 128    # 8 m row-tiles
RT = RROWS // 128    # 16 right row-tiles

AF = mybir.ActivationFunctionType
ALU = mybir.AluOpType
F32 = mybir.dt.float32
BF16 = mybir.dt.bfloat16
FP8 = mybir.dt.bfloat16  # bisect: fp8 off


def _build_body(tc, lblk, rblk, temp, rowsum_o, colsum_o, diag_o):
    nc = tc.nc
    from contextlib import ExitStack

    with ExitStack() as ctx:
        const_pool = ctx.enter_context(tc.tile_pool(name="const", bufs=1))
        small = ctx.enter_context(tc.tile_pool(name="small", bufs=1))
        nat_pool = ctx.enter_context(tc.tile_pool(name="nat", bufs=8))
        sq_pool = ctx.enter_context(tc.tile_pool(name="sq", bufs=2))
        nrm_pool = ctx.enter_context(tc.tile_pool(name="nrm", bufs=4))
        dsc_pool = ctx.enter_context(tc.tile_pool(name="dsc", bufs=2))
        op_pool = ctx.enter_context(tc.tile_pool(name="op", bufs=1))
        e_pool = ctx.enter_context(tc.tile_pool(name="E", bufs=MT))
        ps_mm = ctx.enter_context(tc.tile_pool(name="psmm", bufs=3, space="PSUM"))
        ps_cs = ctx.enter_context(tc.tile_pool(name="pscs", bufs=2, space="PSUM"))

        # ---- constants ----
        eye = const_pool.tile([128, 128], FP8, tag="eye")
        make_identity(nc, eye[:])
        ones_row = const_pool.tile([1, 128], F32, tag="ones_row")
        nc.vector.memset(ones_row[:], 1.0)
        ones_col = const_pool.tile([128, 1], FP8, tag="ones_col")
        nc.vector.memset(ones_col[:], 1.0)
        epsb = const_pool.tile([128, 1], F32, tag="epsb")
        nc.vector.memset(epsb[:], EPS)

        # ---- escale = exp(temperature) broadcast to 128 partitions ----
        t_sb = small.tile([1, 1], F32, tag="t_sb")
        nc.sync.dma_start(t_sb[:], temp.rearrange("(a b) -> a b", a=1))
        esc11 = small.tile([1, 1], F32, tag="esc11")
        nc.scalar.activation(esc11[:], t_sb[:], AF.Exp)
        esc_ps = ps_cs.tile([128, 1], F32, tag="cs", name="escps")
        nc.tensor.matmul(esc_ps[:], ones_row[:], esc11[:], start=True, stop=True)
        escale = small.tile([128, 1], F32, tag="escale")
        nc.vector.tensor_copy(escale[:], esc_ps[:])

        # ---- accumulators / operands ----
        rowacc = small.tile([128, MT * 2], F32, tag="rowacc")
        diagacc = small.tile([128, MT * 2], F32, tag="diagacc")
        colsb = small.tile([1, RROWS], F32, tag="colsb")
        sqL = small.tile([128, MT], F32, tag="sqL")
        sqR = small.tile([128, RT], F32, tag="sqR")
        invL = small.tile([128, MT], F32, tag="invL")
        invR = small.tile([128, RT], F32, tag="invR")
        # L transposed tile-major: lnT[d128, mt, k, row128] (stationary operand)
        lnT = op_pool.tile([128, MT, KT, 128], BF16, tag="lnT")
        # R transposed K-MAJOR: rnT[d128, k, rt, row128] (moving operand:
        # rnT[:, k, 4a:4a+4, :] is a contiguous [128, 512] slice)
        rnT = op_pool.tile([128, KT, RT, 128], BF16, tag="rnT")
        etiles = [e_pool.tile([128, RROWS], FP8, tag="E", name=f"E{m}")
                  for m in range(MT)]

        def load_square(src, sq_all, i, gname):
            nat = nat_pool.tile([128, D], F32, tag="nat", name=f"nat{gname}{i}")
            nc.gpsimd.dma_start(nat[:], src[ts(i, 128), :])
            sqd = sq_pool.tile([128, D], F32, tag="sq", name=f"sq{gname}{i}")
            nc.scalar.activation(sqd[:], nat[:], AF.Square,
                                 accum_out=sq_all[:, ds(i, 1)])
            return nat

        def inv_chain(sq_all, inv_all, a, n, gname):
            # inv = (sq + eps)^-0.5 = exp(-0.5 * ln(sq + eps)); Ln and Exp
            # (and Square) live in one ACT table set -> no table swaps.
            lnt = small.tile([128, n], F32, tag=f"lnt{gname}{a}")
            nc.scalar.activation(lnt[:], sq_all[:, ds(a, n)], AF.Ln,
                                 bias=epsb[:, 0:1])
            nc.scalar.activation(inv_all[:, ds(a, n)], lnt[:], AF.Exp, scale=-0.5)

        def norm_xpose(nat, inv_all, i, dst_slice, gname):
            nrm = nrm_pool.tile([128, D], BF16, tag="nrm", name=f"nrm{gname}{i}")
            nc.vector.tensor_scalar_mul(nrm[:], nat[:], inv_all[:, ds(i, 1)])
            nc.sync.dma_start_transpose(dst_slice, nrm[:])

        def tower_group(src, sq_all, inv_all, is_left, tiles):
            gname = "L" if is_left else "R"
            nats = [load_square(src, sq_all, i, gname) for i in tiles]
            inv_chain(sq_all, inv_all, tiles[0], len(tiles), gname)
            for nat, i in zip(nats, tiles):
                dst = lnT[:, i, :, :] if is_left else rnT[:, :, i, :]
                norm_xpose(nat, inv_all, i, dst, gname)

        # ---- towers, ordered so the first matmuls unblock earliest ----
        tower_group(lblk, sqL, invL, True, [0])
        tower_group(rblk, sqR, invR, False, [0, 1, 2, 3])
        tower_group(rblk, sqR, invR, False, [4, 5, 6, 7])
        tower_group(lblk, sqL, invL, True, [1, 2, 3])
        tower_group(lblk, sqL, invL, True, [4, 5, 6, 7])
        tower_group(rblk, sqR, invR, False, [8, 9, 10, 11])
        tower_group(rblk, sqR, invR, False, [12, 13, 14, 15])

        # ---- matmul stream + fused exp/rowsum + interleaved column sums ----
        cs_tiles = {}

        def colsum_mm(h, mt):
            # cps[h][j] accumulates ones^T @ E over the 8 mt tiles of half h
            for j in range(2):
                if (h, j) not in cs_tiles:
                    cs_tiles[(h, j)] = ps_cs.tile([1, 512], F32, tag="cs",
                                                  name=f"cps{h}_{j}")
                nc.tensor.matmul(
                    cs_tiles[(h, j)][:], ones_col[:],
                    etiles[mt][:, ds(h * 1024 + j * 512, 512)],
                    start=(mt == 0), stop=(mt == MT - 1),
                )

        def colsum_out(h):
            for j in range(2):
                nc.vector.tensor_copy(
                    colsb[:, ds(h * 1024 + j * 512, 512)], cs_tiles[(h, j)][:])

        for h in range(2):
            for mt in range(MT):
                ps = ps_mm.tile([128, 1024], F32, tag="ps", name=f"ps{h}_{mt}")
                for k in range(KT):
                    for j in range(2):
                        nc.tensor.matmul(
                            ps[:, ds(j * 512, 512)],
                            lnT[:, mt, k, :],
                            rnT[:, k, ds((2 * h + j) * 4, 4), :],
                            start=(k == 0), stop=(k == KT - 1),
                        )
                nc.scalar.activation(
                    etiles[mt][:, ds(h * 1024, 1024)], ps[:], AF.Exp,
                    scale=escale[:, 0:1],
                    accum_out=rowacc[:, ds(mt * 2 + h, 1)],
                )
                # colsum matmuls trail the exp by one mt so the PE never
                # stalls on the ScalarE pass; (0,7) lands after (1,0), the
                # h0 PSUM tiles are then copied out to free the cs pool.
                if mt >= 1:
                    colsum_mm(h, mt - 1)
                    if h == 1 and mt == 1:
                        colsum_mm(0, MT - 1)
                        colsum_out(0)
        colsum_mm(1, MT - 1)
        colsum_out(1)

        # ---- diagonal candidates via fused mul+reduce (one DVE pass each) ----
        for h in range(2):
            for mt in range(MT):
                dscr = dsc_pool.tile([128, 128], F32, tag="dscr",
                                     name=f"dscr{h}_{mt}")
                nc.vector.tensor_mul(
                    dscr[:], etiles[mt][:, ds(h * 1024 + mt * 128, 128)], eye[:])
                nc.vector.tensor_reduce(
                    diagacc[:, ds(mt * 2 + h, 1)], dscr[:],
                    axis=mybir.AxisListType.X, op=ALU.add)

        # ---- finalize outputs ----
        rs = small.tile([128, MT], F32, tag="rs")
        nc.vector.tensor_reduce(
            rs[:], rowacc[:].rearrange("p (m c) -> p m c", c=2),
            axis=mybir.AxisListType.X, op=ALU.add,
        )
        nc.sync.dma_start(rowsum_o[:], rs[:])
        nc.sync.dma_start(colsum_o.rearrange("(a c) -> a c", a=1), colsb[:])
        nc.sync.dma_start(diag_o[:], diagacc[:])


_CACHED = {}


def _get_program():
    if "nc" in _CACHED:
        return _CACHED["nc"]
    nc = bacc.Bacc("TRN2", target_bir_lowering=False, debug=False,
                   num_devices=PGRID * QGRID)
    lblk = nc.dram_tensor("lblk", [LROWS, D], F32, kind="ExternalInput").ap()
    rblk = nc.dram_tensor("rblk", [RROWS, D], F32, kind="ExternalInput").ap()
    temp = nc.dram_tensor("temp", [1], F32, kind="ExternalInput").ap()
    rowsum_o = nc.dram_tensor("rowsum", [128, MT], F32, kind="ExternalOutput").ap()
    colsum_o = nc.dram_tensor("colsum", [RROWS], F32, kind="ExternalOutput").ap()
    diag_o = nc.dram_tensor("diag", [128, MT * 2], F32, kind="ExternalOutput").ap()
    with tile.TileContext(nc) as tc:
        _build_body(tc, lblk, rblk, temp, rowsum_o, colsum_o, diag_o)
    nc.compile()
    _CACHED["nc"] = nc
    return nc


def _run(inputs, trace=False):
    from concourse.bass_utils import run_bass_kernel_spmd

    nc = _get_program()
    left = np.ascontiguousarray(inputs["left"], dtype=np.float32)
    right = np.ascontiguousarray(inputs["right"], dtype=np.float32)
    temp = np.ascontiguousarray(inputs["temperature"], dtype=np.float32)

    in_maps = []
    for p in range(PGRID):
        for q in range(QGRID):
            in_maps.append({
                "lblk": left[p * LROWS:(p + 1) * LROWS],
                "rblk": right[q * RROWS:(q + 1) * RROWS],
                "temp": temp,
            })
    res = run_bass_kernel_spmd(nc, in_maps, core_ids=list(range(PGRID * QGRID)),
                               trace=trace)
    return res


def _combine(results):
    rowsum = np.zeros(B, dtype=np.float64)
    colsum = np.zeros(B, dtype=np.float64)
    diag = np.zeros(B, dtype=np.float64)
    for p in range(PGRID):
        for q in range(QGRID):
            r = results[p * QGRID + q]
            rs = r["rowsum"].astype(np.float64)  # [128, MT]
            rowsum[p * LROWS:(p + 1) * LROWS] += rs.T.reshape(-1)
            colsum[q * RROWS:(q + 1) * RROWS] += r["colsum"].astype(np.float64)
            delta = LROWS * p - RROWS * q
            if delta in (0, 1024):
                a = delta // 1024
                d = r["diag"].astype(np.float64).reshape(128, MT, 2)[:, :, a]
                diag[p * LROWS:(p + 1) * LROWS] = d.T.reshape(-1)
    tr_l = float(np.sum(diag / rowsum))
    tr_r = float(np.sum(diag / colsum))
    log_eps = math.log(EPS)
    log_1meps = math.log(1.0 - EPS)
    loss_l = -(tr_l * log_1meps + (B - tr_l) * log_eps)
    loss_r = -(tr_r * log_1meps + (B - tr_r) * log_eps)
    loss = WEIGHT * (loss_l + loss_r) / 2.0 / B
    return np.asarray(loss, dtype=np.float32)


def kernel(**inputs):
    res = _run(inputs, trace=False)
    return _combine(res.results)


def kernel_traced(**inputs):
    res = _run(inputs, trace=True)
    return _combine(res.results), res


# revision 6
# speedup vs baseline: 1.0320x; 1.0320x over previous
"""CLIP-style contrastive loss on 8 Trainium2 NeuronCores.

Math: with labels = eye(B), the Keras CCE(prob, eye) loss only depends on the
diagonal of the softmax probabilities:
    sum_ij prob_ij * logclip_ij = tr * log(1-eps) + (B - tr) * log(eps)
where tr = trace(prob); prob_ii = E_ii / rowsum(E) with
E = exp(scale * l2norm(left) @ l2norm(right).T).

Sharding: 4x2 grid over the BxB similarity matrix. Core (p, q) owns
left rows [1024p, 1024p+1024) x right rows [2048q, 2048q+2048).

Per-core pipeline (v3):
  - 24 row-tile loads cast fp32->bf16 in the DMA (SWDGE), square+sqsum on
    ScalarE (Square and Exp share the exp_and_others ACT table set -> a
    single table load; the old Sqrt/Ln variants thrashed 7-15 loads).
  - inv-norms via Quake rsqrt (bit trick + 1 Newton step) entirely on DVE.
  - LEFT tower is transposed RAW (un-normalized): its 1/||l|| folds into the
    exp's per-partition scale, so L transposes depend only on the load.
    RIGHT tower normalizes (bf16 4x tensor_scalar) before its transpose.
  - one xbar DMA transpose PER ROW TILE: lnT tile-major (stationary),
    rnT K-MAJOR so each matmul's moving operand is a contiguous [128,512].
  - MM stream h -> mt -> k -> j with a 3-deep [128,1024] PSUM rotation;
    tower groups are interleaved into the MM issue order so ScalarE's
    in-order queue reaches each exp pass just as its PSUM tile is ready.
  - exp+rowsum fused (accum_out), E stored fp8e4; ones-vector fp8 colsum
    matmuls trail one mt behind; diagonal via eye-mask mul+reduce.
Host combines partial sums (O(B) work only).
"""

import math
import numpy as np

import concourse.bass as bass
import concourse.mybir as mybir
import concourse.tile as tile
from concourse import bacc
from concourse.bass import ds, ts
from concourse.masks import make_identity

B = 4096
D = 1024
EPS = 1e-7
WEIGHT = 1.0

PGRID = 4  # row groups (left)
QGRID = 2  # col groups (right)
LROWS = B // PGRID   # 1024 left rows per core
RROWS = B // QGRID   # 2048 right rows per core
KT = D // 128        # 8 contraction k-tiles
MT = LROWS // 128    # 8 m row-tiles
RT = RROWS // 128    # 16 right row-tiles

AF = mybir.ActivationFunctionType
ALU = mybir.AluOpType
F32 = mybir.dt.float32
BF16 = mybir.dt.bfloat16
FP8 = mybir.dt.float8e4
I32 = mybir.dt.int32

QMAGIC = 0x5F3759DF


def _build_body(tc, lblk, rblk, temp, rowsum_o, colsum_o, diag_o):
    nc = tc.nc
    from contextlib import ExitStack

    with ExitStack() as ctx:
        const_pool = ctx.enter_context(tc.tile_pool(name="const", bufs=1))
        small = ctx.enter_context(tc.tile_pool(name="small", bufs=1))
        nat_pool = ctx.enter_context(tc.tile_pool(name="nat", bufs=10))
        nrm_pool = ctx.enter_context(tc.tile_pool(name="nrm", bufs=4))
        sq_pool = ctx.enter_context(tc.tile_pool(name="sq", bufs=2))
        dsc_pool = ctx.enter_context(tc.tile_pool(name="dsc", bufs=2))
        op_pool = ctx.enter_context(tc.tile_pool(name="op", bufs=1))
        e_pool = ctx.enter_context(tc.tile_pool(name="E", bufs=MT))
        ps_mm = ctx.enter_context(tc.tile_pool(name="psmm", bufs=3, space="PSUM"))
        ps_cs = ctx.enter_context(tc.tile_pool(name="pscs", bufs=2, space="PSUM"))

        # ---- constants ----
        eye = const_pool.tile([128, 128], BF16, tag="eye")
        make_identity(nc, eye[:])
        ones_row = const_pool.tile([1, 128], F32, tag="ones_row")
        nc.vector.memset(ones_row[:], 1.0)
        ones_col = const_pool.tile([128, 1], FP8, tag="ones_col")
        nc.vector.memset(ones_col[:], 1.0)
        magic = const_pool.tile([128, 4], I32, tag="magic")
        nc.vector.memset(magic[:], QMAGIC)

        # ---- escale = exp(temperature) broadcast to 128 partitions ----
        t_sb = small.tile([1, 1], F32, tag="t_sb")
        nc.sync.dma_start(t_sb[:], temp.rearrange("(a b) -> a b", a=1))
        esc11 = small.tile([1, 1], F32, tag="esc11")
        nc.scalar.activation(esc11[:], t_sb[:], AF.Exp)
        esc_ps = ps_cs.tile([128, 1], F32, tag="cs", name="escps")
        nc.tensor.matmul(esc_ps[:], ones_row[:], esc11[:], start=True, stop=True)
        escale = small.tile([128, 1], F32, tag="escale")
        nc.vector.tensor_copy(escale[:], esc_ps[:])

        # ---- accumulators / operands ----
        rowacc = small.tile([128, MT * 2], F32, tag="rowacc")
        diagacc = small.tile([128, MT * 2], F32, tag="diagacc")
        colsb = small.tile([1, RROWS], F32, tag="colsb")
        sqL = small.tile([128, MT], F32, tag="sqL")
        sqR = small.tile([128, RT], F32, tag="sqR")
        invL = small.tile([128, MT], F32, tag="invL")
        invR = small.tile([128, RT], F32, tag="invR")
        einv = small.tile([128, MT], F32, tag="einv")
        # L transposed tile-major, RAW bf16: lnT[d128, mt, k, row128]
        lnT = op_pool.tile([128, MT, KT, 128], BF16, tag="lnT")
        # R transposed K-MAJOR, normalized: rnT[d128, k, rt, row128]
        rnT = op_pool.tile([128, KT, RT, 128], BF16, tag="rnT")
        etiles = [e_pool.tile([128, RROWS], FP8, tag="E", name=f"E{m}")
                  for m in range(MT)]

        def load_square(src, sq_all, i, gname):
            nat = nat_pool.tile([128, D], BF16, tag="nat", name=f"nat{gname}{i}")
            nc.gpsimd.dma_start(nat[:], src[ts(i, 128), :])  # fp32->bf16 cast
            sqd = sq_pool.tile([128, D], BF16, tag="sq", name=f"sq{gname}{i}")
            nc.scalar.activation(sqd[:], nat[:], AF.Square,
                                 accum_out=sq_all[:, ds(i, 1)])
            return nat

        def rsqrt_chain(sq_all, inv_all, a, n, gname):
            # Quake inverse sqrt: y0 = bits(QMAGIC - (bits(x) >> 1)),
            # one Newton step y1 = y0*(1.5 - 0.5*x*y0^2).  All on DVE, so
            # ScalarE keeps a single ACT table set. rel err <= ~2e-3.
            x = sq_all[:, ds(a, n)]
            sh = small.tile([128, n], I32, tag=f"sh{gname}{a}")
            nc.vector.tensor_scalar(sh[:], x.bitcast(I32), 1, None,
                                    op0=ALU.logical_shift_right)
            y0 = small.tile([128, n], F32, tag=f"y0{gname}{a}")
            nc.vector.tensor_tensor(y0[:].bitcast(I32), magic[:, 0:n], sh[:],
                                    op=ALU.subtract)
            t1 = small.tile([128, n], F32, tag=f"t1{gname}{a}")
            nc.vector.tensor_tensor(t1[:], y0[:], y0[:], op=ALU.mult)
            t2 = small.tile([128, n], F32, tag=f"t2{gname}{a}")
            nc.vector.tensor_tensor(t2[:], t1[:], x, op=ALU.mult)
            t3 = small.tile([128, n], F32, tag=f"t3{gname}{a}")
            nc.vector.tensor_scalar(t3[:], t2[:], -0.5, 1.5, op0=ALU.mult,
                                    op1=ALU.add)
            nc.vector.tensor_tensor(inv_all[:, ds(a, n)], y0[:], t3[:],
                                    op=ALU.mult)

        def tower_group(src, sq_all, inv_all, is_left, tiles):
            gname = "L" if is_left else "R"
            nats = [load_square(src, sq_all, i, gname) for i in tiles]
            if is_left:
                # RAW transpose; 1/||l|| folds into the exp scale later.
                for nat, i in zip(nats, tiles):
                    nc.sync.dma_start_transpose(lnT[:, i, :, :], nat[:])
                rsqrt_chain(sq_all, inv_all, tiles[0], len(tiles), gname)
            else:
                rsqrt_chain(sq_all, inv_all, tiles[0], len(tiles), gname)
                for nat, i in zip(nats, tiles):
                    nrm = nrm_pool.tile([128, D], BF16, tag="nrm",
                                        name=f"nrm{gname}{i}")
                    nc.vector.tensor_scalar_mul(nrm[:], nat[:],
                                                inv_all[:, ds(i, 1)])
                    nc.sync.dma_start_transpose(rnT[:, :, i, :], nrm[:])

        # ---- matmul block + fused exp/rowsum + trailing column sums ----
        cs_tiles = {}

        def colsum_mm(h, mt):
            for j in range(2):
                if (h, j) not in cs_tiles:
                    cs_tiles[(h, j)] = ps_cs.tile([1, 512], F32, tag="cs",
                                                  name=f"cps{h}_{j}")
                nc.tensor.matmul(
                    cs_tiles[(h, j)][:], ones_col[:],
                    etiles[mt][:, ds(h * 1024 + j * 512, 512)],
                    start=(mt == 0), stop=(mt == MT - 1),
                )

        def colsum_out(h):
            for j in range(2):
                nc.vector.tensor_copy(
                    colsb[:, ds(h * 1024 + j * 512, 512)], cs_tiles[(h, j)][:])

        def mm_block(h, mt):
            ps = ps_mm.tile([128, 1024], F32, tag="ps", name=f"ps{h}_{mt}")
            for k in range(KT):
                for j in range(2):
                    nc.tensor.matmul(
                        ps[:, ds(j * 512, 512)],
                        lnT[:, mt, k, :],
                        rnT[:, k, ds((2 * h + j) * 4, 4), :],
                        start=(k == 0), stop=(k == KT - 1),
                    )
            nc.scalar.activation(
                etiles[mt][:, ds(h * 1024, 1024)], ps[:], AF.Exp,
                scale=einv[:, ds(mt, 1)],
                accum_out=rowacc[:, ds(mt * 2 + h, 1)],
            )
            if mt >= 1:
                colsum_mm(h, mt - 1)
                if h == 1 and mt == 1:
                    colsum_mm(0, MT - 1)
                    colsum_out(0)

        # ---- interleaved issue schedule ----
        # Towers threaded between MM blocks so ScalarE's in-order queue
        # reaches exp(h,mt) right as its PSUM tile completes.
        tower_group(lblk, sqL, invL, True, [0])
        tower_group(rblk, sqR, invR, False, [0, 1, 2, 3])
        tower_group(rblk, sqR, invR, False, [4, 5, 6, 7])
        tower_group(lblk, sqL, invL, True, [1, 2, 3])
        # einv[:, mt] = escale * invL[:, mt] (per-partition exp scale); done
        # in two halves so mt=0..3 scales are ready before sqL4-7 lands.
        nc.vector.tensor_scalar_mul(einv[:, 0:4], invL[:, 0:4], escale[:, 0:1])

        mm_block(0, 0)
        tower_group(lblk, sqL, invL, True, [4, 5, 6, 7])
        nc.vector.tensor_scalar_mul(einv[:, 4:8], invL[:, 4:8], escale[:, 0:1])
        mm_block(0, 1)
        mm_block(0, 2)
        tower_group(rblk, sqR, invR, False, [8, 9, 10, 11])
        mm_block(0, 3)
        mm_block(0, 4)
        tower_group(rblk, sqR, invR, False, [12, 13, 14, 15])
        for mt in range(5, MT):
            mm_block(0, mt)
        for mt in range(MT):
            mm_block(1, mt)
        colsum_mm(1, MT - 1)
        colsum_out(1)

        # ---- diagonal candidates ----
        for h in range(2):
            for mt in range(MT):
                dscr = dsc_pool.tile([128, 128], BF16, tag="dscr",
                                     name=f"dscr{h}_{mt}")
                nc.vector.tensor_mul(
                    dscr[:], etiles[mt][:, ds(h * 1024 + mt * 128, 128)], eye[:])
                nc.vector.tensor_reduce(
                    diagacc[:, ds(mt * 2 + h, 1)], dscr[:],
                    axis=mybir.AxisListType.X, op=ALU.add)

        # ---- finalize outputs ----
        rs = small.tile([128, MT], F32, tag="rs")
        nc.vector.tensor_reduce(
            rs[:], rowacc[:].rearrange("p (m c) -> p m c", c=2),
            axis=mybir.AxisListType.X, op=ALU.add,
        )
        nc.sync.dma_start(rowsum_o[:], rs[:])
        nc.sync.dma_start(colsum_o.rearrange("(a c) -> a c", a=1), colsb[:])
        nc.sync.dma_start(diag_o[:], diagacc[:])


_CACHED = {}


def _get_program():
    if "nc" in _CACHED:
        return _CACHED["nc"]
    nc = bacc.Bacc("TRN2", target_bir_lowering=False, debug=False,
                   num_devices=PGRID * QGRID)
    lblk = nc.dram_tensor("lblk", [LROWS, D], F32, kind="ExternalInput").ap()
    rblk = nc.dram_tensor("rblk", [RROWS, D], F32, kind="ExternalInput").ap()
    temp = nc.dram_tensor("temp", [1], F32, kind="ExternalInput").ap()
    rowsum_o = nc.dram_tensor("rowsum", [128, MT], F32, kind="ExternalOutput").ap()
    colsum_o = nc.dram_tensor("colsum", [RROWS], F32, kind="ExternalOutput").ap()
    diag_o = nc.dram_tensor("diag", [128, MT * 2], F32, kind="ExternalOutput").ap()
    with tile.TileContext(nc) as tc:
        _build_body(tc, lblk, rblk, temp, rowsum_o, colsum_o, diag_o)
    nc.compile()
    _CACHED["nc"] = nc
    return nc


def _run(inputs, trace=False):
    from concourse.bass_utils import run_bass_kernel_spmd

    nc = _get_program()
    left = np.ascontiguousarray(inputs["left"], dtype=np.float32)
    right = np.ascontiguousarray(inputs["right"], dtype=np.float32)
    temp = np.ascontiguousarray(inputs["temperature"], dtype=np.float32)

    in_maps = []
    for p in range(PGRID):
        for q in range(QGRID):
            in_maps.append({
                "lblk": left[p * LROWS:(p + 1) * LROWS],
                "rblk": right[q * RROWS:(q + 1) * RROWS],
                "temp": temp,
            })
    res = run_bass_kernel_spmd(nc, in_maps, core_ids=list(range(PGRID * QGRID)),
                               trace=trace)
    return res


def _combine(results):
    rowsum = np.zeros(B, dtype=np.float64)
    colsum = np.zeros(B, dtype=np.float64)
    diag = np.zeros(B, dtype=np.float64)
    for p in range(PGRID):
        for q in range(QGRID):
            r = results[p * QGRID + q]
            rs = r["rowsum"].astype(np.float64)  # [128, MT]
            rowsum[p * LROWS:(p + 1) * LROWS] += rs.T.reshape(-1)
            colsum[q * RROWS:(q + 1) * RROWS] += r["colsum"].astype(np.float64)
            delta = LROWS * p - RROWS * q
            if delta in (0, 1024):
                a = delta // 1024
                d = r["diag"].astype(np.float64).reshape(128, MT, 2)[:, :, a]
                diag[p * LROWS:(p + 1) * LROWS] = d.T.reshape(-1)
    tr_l = float(np.sum(diag / rowsum))
    tr_r = float(np.sum(diag / colsum))
    log_eps = math.log(EPS)
    log_1meps = math.log(1.0 - EPS)
    loss_l = -(tr_l * log_1meps + (B - tr_l) * log_eps)
    loss_r = -(tr_r * log_1meps + (B - tr_r) * log_eps)
    loss = WEIGHT * (loss_l + loss_r) / 2.0 / B
    return np.asarray(loss, dtype=np.float32)


def kernel(**inputs):
    res = _run(inputs, trace=False)
    return _combine(res.results)


def kernel_traced(**inputs):
    res = _run(inputs, trace=True)
    return _combine(res.results), res
